# revision 57
# baseline (speedup 1.0000x reference)
"""Causal self-attention (RoPE) Trainium2 kernel, 8-way head-parallel.

Sharding: each of the 8 cores computes 2 of the 16 heads for all 4 batches
(tensor parallel over heads: W_qkv column-split, W_proj row-split). Host
pre-transposes x -> xT [C, B*T], slices per-core weights, and sum-reduces the
8 partial projection outputs (+ b_proj) — the standard row-parallel TP reduce.

Per-core dataflow (fp32 storage, bf16 matmuls):
  qkvT = W_slice.T @ xT            [feat, tok] PSUM, bias added on evac
  RoPE on qT,kT                    (rotate-half via SBUF partition-move DMAs)
  v: PE-transpose vT -> v tiles    [tok, d] (+ ones column for softmax denom)
  per (b, h, i-chunk):  S^T tiles = kT_jtile.T @ qT_ichunk   (j on partitions)
      two heads' S matmuls emitted as adjacent pairs -> PE row-tile
      concurrency ((0,0)/(64,0), K=64 each)
      P^T = exp(S^T/8) (ACT), causal mask on diagonal tiles (multiplicative,
      batched [128,1024] ops)
      [yT_h | denom] += v_aug.T @ P^T   accumulated over j-tiles in PSUM
  per i-chunk: denom -> gpsimd partition_broadcast -> DVE approx-reciprocal,
      yT merged via col-tiled identity matmuls, scaled, projected with
      N=1024 bf16-PSUM matmuls, written straight to outp rows.
"""

import numpy as np

import concourse.bass as bass
import concourse.mybir as mybir
import concourse.tile as tile

F32 = mybir.dt.float32
F32R = mybir.dt.float32r
BF16 = mybir.dt.bfloat16
AF = mybir.ActivationFunctionType
OP = mybir.AluOpType

# ---------------------------------------------------------------- tile patch
# This walrus build rejects >1 embedded sync-wait on sync-engine CTRL
# instructions; Tile's tail drain embeds one wait per outstanding semaphore.
# Split them across NOPs (1 wait each) before the drain.


def _patched_drain_and_barrier(self, tick_clock, wait_clock):
    from concourse.tile import ScopedClock

    nc = self.nc
    probe = nc.sync.nop(nofuse=True)
    wait_clock.add_sem_waits(probe.ins, ScopedClock({None: tick_clock.global_clock}))
    si = probe.ins.sync_info
    waits = list(si.on_wait) if si is not None and si.on_wait else []
    if len(waits) > 1:
        si.on_wait = waits[:1]
        for w in waits[1:]:
            nop = nc.sync.nop(nofuse=True)
            nsi = nop.ins.sync_info
            if nsi is None:
                nop.ins.sync_info = mybir.SyncInfo(on_wait=[w], on_update=[])
            else:
                nsi.on_wait = [w]
    nc.sync.drain()
    nc.all_engine_barrier()
    assert self.sems is not None
    popped = nc._tile_sem_poison_stack.pop()
    assert popped is self._sem_poison
    # chunk the sem clears: the range-encoded gpsimd drain (dma_reset) in this
    # walrus build rejects wide semaphore ranges ("ISA wrong length")
    sems = sorted(
        s.num if hasattr(s, "num") else s for s in self.sems.allocated().values()
    )
    for i in range(0, len(sems), 16):
        nc.clear_and_free_semaphores(sems[i : i + 16])
    nc.all_engine_barrier()


tile.TileContext._drain_and_barrier = _patched_drain_and_barrier


def _split_waits(nc):
    """Hoist all but one sync-wait per instruction onto same-engine NOPs
    (this walrus codegen supports a single embedded wait per instruction)."""
    n = 0
    for f in nc.m.functions:
        for blk in f.blocks:
            out = []
            changed = False
            for ins in blk.instructions:
                si = ins.sync_info
                if si is not None and si.on_wait and len(si.on_wait) > 1:
                    waits = list(si.on_wait)
                    for w in waits[:-1]:
                        n += 1
                        out.append(
                            mybir.InstNoOp(
                                name=f"wsplit{n}",
                                engine=ins.engine,
                                sync_info=mybir.SyncInfo(on_wait=[w], on_update=[]),
                                bass_nofuse=True,
                            )
                        )
                    si.on_wait = waits[-1:]
                    changed = True
                out.append(ins)
            if changed:
                blk.instructions = out
    return n

# ------------------------------------------------------------------- builder

HD = 64  # head dim (fixed)
ROPE_BASE = 10000.0


def build_nc(B, T, C, rope_k_engine="gpsimd", mask_engine="vector", split=True):
    """One core's program: 2 heads x B batches. T % 512 == 0, C % 128 == 0."""
    assert T % 512 == 0 and C % 128 == 0
    TOK = B * T
    KC = C // 128   # contraction chunks for QKV
    NCH = T // 512  # i-chunks per batch
    NJT = T // 128  # j-tiles per batch
    FW = 128        # qkv feature width per tensor (2 heads * 64)

    nc = bass.Bass()
    # x pre-arranged host-side as [global chunk, partition, KC*512] so each
    # chunk DMA is 128 x 8KB contiguous lines; weights as [128, KC*FW].
    xTc = nc.dram_tensor("xTc", [TOK // 512, 128, KC, 512], BF16, kind="ExternalInput")
    wq = nc.dram_tensor("wq", [128, KC * FW], BF16, kind="ExternalInput")
    wk = nc.dram_tensor("wk", [128, KC * FW], BF16, kind="ExternalInput")
    wv = nc.dram_tensor("wv", [128, KC * FW], BF16, kind="ExternalInput")
    bq = nc.dram_tensor("bq", [FW, 1], F32, kind="ExternalInput")
    bk = nc.dram_tensor("bk", [FW, 1], F32, kind="ExternalInput")
    bv = nc.dram_tensor("bv", [FW, 1], F32, kind="ExternalInput")
    wp = nc.dram_tensor("wp", [FW, C], BF16, kind="ExternalInput")
    cosT = nc.dram_tensor("cosT", [FW, T], BF16, kind="ExternalInput")
    sinT = nc.dram_tensor("sinT", [FW, T], BF16, kind="ExternalInput")
    mask4 = nc.dram_tensor("mask4", [128, 4 * 1024], BF16, kind="ExternalInput")
    outp = nc.dram_tensor("outp", [TOK, C], BF16, kind="ExternalOutput")

    def xchunk(gcn):
        return xTc[gcn : gcn + 1, :, :, :].rearrange("g p a f -> p (g a) f")

    rope_eng = {"vector": nc.vector, "gpsimd": nc.gpsimd}[rope_k_engine]
    mask_eng = {"vector": nc.vector, "gpsimd": nc.gpsimd}[mask_engine]

    with tile.TileContext(nc) as tc:
        with (
            tc.tile_pool(name="const", bufs=1) as cpool,
            tc.tile_pool(name="xt", bufs=3) as xpool,
            tc.tile_pool(name="qk", bufs=2) as qkpool,
            tc.tile_pool(name="vv", bufs=2) as vpool,
            tc.tile_pool(name="yy", bufs=2) as ypool,
            tc.tile_pool(name="small", bufs=2) as spool,
            tc.tile_pool(name="den", bufs=3) as dpool,
            tc.tile_pool(name="bc", bufs=2) as bcpool,
            tc.tile_pool(name="pt", bufs=4) as ptpool,
            tc.tile_pool(name="outs", bufs=4) as opool,
            tc.tile_pool(name="dram", bufs=2, space="DRAM") as drampool,
            tc.tile_pool(name="ps_mm", bufs=2, space="PSUM") as ps_mm,
            tc.tile_pool(name="ps_s", bufs=2, space="PSUM") as ps_s,
            tc.tile_pool(name="ps_y", bufs=1, space="PSUM") as ps_y,
        ):
            # ---- constants (small ones first; the big cos/sin/wp2 loads are
            # emitted after the first xt prefetches so they don't delay the
            # first QKV matmuls in the DMA queues) ----
            w_sb = {}
            for name, dram in (("wq", wq), ("wk", wk), ("wv", wv)):
                t = cpool.tile([128, KC, FW], BF16, tag=name)
                nc.sync.dma_start(
                    t[:, :, :], dram[:, :].rearrange("p (a f) -> p a f", a=KC)
                )
                w_sb[name] = t
            b_sb = {}
            for name, dram in (("bq", bq), ("bk", bk), ("bv", bv)):
                t = cpool.tile([FW, 1], F32, tag=name)
                nc.sync.dma_start(t[:, :], dram[:, :])
                b_sb[name] = t
            ident = cpool.tile([128, 128], BF16, tag="ident")
            from concourse.masks import make_identity

            make_identity(nc, ident[:, :])
            ones_raw = cpool.tile([128, 128], F32, tag="ones_raw")
            nc.vector.memset(ones_raw[:, :], 1.0)

            # prefetch the first batch's x chunks ahead of the big constants;
            # the first two are split across several DMA queues so the very
            # first QKV matmul isn't gated on a single-queue 1MB transfer
            xt_pre = {}
            for cn in range(min(NCH, 3)):
                xt = xpool.tile([128, KC, 512], BF16, tag="xt", name=f"xt0_{cn}")
                src = xchunk(cn)
                if cn < 2:
                    for q in range(4):
                        a0 = q * (KC // 4)
                        a1 = a0 + KC // 4
                        nc.sync.dma_start(xt[:, a0:a1, :], src[:, a0:a1, :])
                else:
                    nc.sync.dma_start(xt[:, :, :], src)
                xt_pre[cn] = xt

            cos_sb = cpool.tile([FW, T], BF16, tag="cos")
            nc.sync.dma_start(cos_sb[:, :], cosT[:, :])
            sin_sb = cpool.tile([FW, T], BF16, tag="sin")
            nc.sync.dma_start(sin_sb[:, :], sinT[:, :])
            mask_sb = cpool.tile([128, 4 * 1024], BF16, tag="mask")
            nc.sync.dma_start(mask_sb[:, :], mask4[:, :])
            wp2_sb = cpool.tile([FW, C], BF16, tag="wp2")
            nc.sync.dma_start(wp2_sb[:, :], wp[:, :])

            state = {}

            def alloc_qkv(b):
                st = state.setdefault(b, {})
                st["qT"] = qkpool.tile([FW, T], BF16, tag="qT", name=f"qT{b}")
                st["kT"] = qkpool.tile([FW, T], BF16, tag="kT", name=f"kT{b}")
                st["v0"] = vpool.tile([128, NJT, HD + 1], BF16, tag="v0", name=f"v0{b}")
                st["v1"] = vpool.tile([128, NJT, HD + 1], BF16, tag="v1", name=f"v1{b}")
                nc.vector.tensor_copy(
                    st["v0"][:, :, HD], ones_raw[:, 0:1].broadcast_to([128, NJT])
                )
                nc.vector.tensor_copy(
                    st["v1"][:, :, HD], ones_raw[:, 0:1].broadcast_to([128, NJT])
                )

            def emit_rope(b, cn, name, ps):
                st = state[b]
                ts0 = cn * 512
                dest = st["qT"] if name == "wq" else st["kT"]
                dch = dest[:, ts0 : ts0 + 512]
                bias = b_sb["bq" if name == "wq" else "bk"]
                nc.vector.tensor_scalar_add(dch, ps[:, :], bias[:, :])
                # rotate-half as 4 SBUF->SBUF DMA partition moves (sign is
                # baked into sinT); no PE/PSUM involved
                swp = spool.tile([128, 512], BF16, tag="swp", name=f"swp{b}_{cn}_{name}")
                for r0, r1 in ((0, 32), (32, 0), (64, 96), (96, 64)):
                    nc.sync.dma_start(
                        swp[r0 : r0 + 32, :], dch[r1 : r1 + 32, :]
                    )
                cc = cos_sb[:, ts0 : ts0 + 512]
                ss = sin_sb[:, ts0 : ts0 + 512]
                t1 = spool.tile([128, 512], F32, tag="t1", name=f"t1{b}_{cn}_{name}")
                t2 = spool.tile([128, 512], F32, tag="t2", name=f"t2{b}_{cn}_{name}")
                # b0's chunks run before attention exists: DVE is the pacing
                # engine there, so push the q-rope to gpsimd as well
                eng = nc.vector if (name == "wq" and b != 0) else rope_eng
                eng.tensor_tensor(t1[:, :], dch, cc, op=OP.mult)
                nc.vector.tensor_tensor(t2[:, :], swp[:, :], ss, op=OP.mult)
                eng.tensor_tensor(dch, t1[:, :], t2[:, :], op=OP.add)

            def emit_qkv_chunk(b, cn):
                st = state[b]
                v0, v1 = st["v0"], st["v1"]
                gcn = b * NCH + cn
                if b == 0 and cn in xt_pre:
                    xt = xt_pre.pop(cn)
                else:
                    xt = xpool.tile([128, KC, 512], BF16, tag="xt", name=f"xt{b}_{cn}")
                    nc.sync.dma_start(xt[:, :, :], xchunk(gcn))
                pss = {}
                for name in ("wq", "wk", "wv"):
                    ps = ps_mm.tile([128, 512], F32, tag="mm", name=f"qkvps{b}_{cn}_{name}")
                    for kc in range(KC):
                        nc.tensor.matmul(
                            ps[:, :],
                            lhsT=w_sb[name][:, kc, :],
                            rhs=xt[:, kc, :],
                            start=(kc == 0),
                            stop=(kc == KC - 1),
                        )
                    pss[name] = ps
                    if name == "wq":
                        # emitted after the wk chain is queued so the DVE evac
                        # overlaps PE work
                        pass
                    elif name == "wk":
                        emit_rope(b, cn, "wq", pss["wq"])
                    else:
                        emit_rope(b, cn, "wk", pss["wk"])
                ps = pss["wv"]
                vch = spool.tile([128, 512], BF16, tag="vch", name=f"vch{b}_{cn}")
                nc.vector.tensor_scalar_add(vch[:, :], ps[:, :], b_sb["bv"][:, :])
                # all four 128x128 transposes land in one PSUM tile so the v
                # evacuation is two wide strided copies
                pstw = ps_mm.tile([128, 512], BF16, tag="mm", name=f"pstw{b}_{cn}")
                for qd in range(4):
                    nc.tensor.transpose(
                        pstw[:, qd * 128 : qd * 128 + 128],
                        vch[:, qd * 128 : qd * 128 + 128],
                        ident[:, :],
                    )
                psv = pstw[:, :].rearrange("p (t f) -> p t f", t=4)
                nc.vector.tensor_copy(
                    v0[:, cn * 4 : cn * 4 + 4, 0:HD], psv[:, :, 0:HD]
                )
                nc.vector.tensor_copy(
                    v1[:, cn * 4 : cn * 4 + 4, 0:HD], psv[:, :, HD:128]
                )

            def alloc_attn(b):
                st = state[b]
                st["yT0"] = ypool.tile([HD, T], BF16, tag="yT0", name=f"yT0{b}")
                st["yT1"] = ypool.tile([HD, T], BF16, tag="yT1", name=f"yT1{b}")

            def emit_attn_ic(b, ic):
                st = state[b]
                qT, kT = st["qT"], st["kT"]
                vh = {0: st["v0"], 1: st["v1"]}
                yTh = {0: st["yT0"], 1: st["yT1"]}
                i0 = ic * 512
                njt = (ic + 1) * 4
                yps = {}
                for h in range(2):
                    yps[h] = ps_y.tile([HD + 1, 512], F32, tag=f"y{h}", name=f"yps{b}_{ic}_{h}")

                def emit_pv(jt, ptt):
                    for h in range(2):
                        nc.tensor.matmul(
                            yps[h][:, :],
                            lhsT=vh[h][:, jt, :],
                            rhs=ptt[:, h * 512 : h * 512 + 512],
                            start=(jt == 0),
                            stop=(jt == njt - 1),
                            skip_group_check=True,
                        )

                # Both heads' S for one j-tile share one [128,1024] sps tile
                # (two PSUM banks): the pair-mates then have the same
                # readiness gate, issue adjacently, and run concurrently on
                # PE row-tiles (0,0)/(64,0).
                for jt in range(njt):
                    sp = ps_s.tile([128, 1024], F32, tag="s", name=f"sps{b}_{ic}_{jt}")
                    for h in range(2):
                        hr0 = h * HD
                        nc.tensor.matmul(
                            sp[:, h * 512 : h * 512 + 512],
                            lhsT=kT[hr0 : hr0 + HD, jt * 128 : jt * 128 + 128],
                            rhs=qT[hr0 : hr0 + HD, i0 : i0 + 512],
                            start=True,
                            stop=True,
                        )
                    ptt = ptpool.tile([128, 1024], BF16, tag="pt", name=f"pt{b}_{ic}_{jt}")
                    nc.scalar.activation(
                        ptt[:, :],
                        sp[:, :],
                        AF.Exp,
                        scale=float(1.0 / np.sqrt(HD)),
                    )
                    dv = jt - (njt - 4)
                    if dv >= 0:  # diagonal tile: mask both heads at once
                        mask_eng.tensor_tensor(
                            ptt[:, :],
                            ptt[:, :],
                            mask_sb[:, dv * 1024 : dv * 1024 + 1024],
                            op=OP.mult,
                        )
                    emit_pv(jt, ptt)
                dts = {}
                for h in range(2):
                    nc.scalar.activation(
                        yTh[h][:, i0 : i0 + 512], yps[h][0:HD, :], AF.Copy
                    )
                    den_t = dpool.tile([HD + 1, 512], F32, tag="den", name=f"den{b}_{ic}_{h}")
                    nc.vector.tensor_copy(den_t[HD : HD + 1, :], yps[h][HD : HD + 1, :])
                    dts[h] = den_t
                st[("den", ic)] = dts

            def emit_recip_ic(b, ic):
                # reciprocal computed in a DMA-transposed [128, 8] layout so
                # all DVE lanes work (the natural [2, 512] layout would leave
                # 126 lanes idle for the slow iterative-divide op), then
                # DMA-broadcast from DRAM across the 64 partitions per head.
                st = state[b]
                dts = st[("den", ic)]
                dd = drampool.tile([2, 512], F32, tag="dd", name=f"dd{b}_{ic}")
                rd = drampool.tile([2, 512], F32, tag="rd", name=f"rd{b}_{ic}")
                for h in range(2):
                    nc.sync.dma_start(dd[h : h + 1, :], dts[h][HD : HD + 1, :])
                dtr = bcpool.tile([128, 2, 4], F32, tag="dtr", name=f"dtr{b}_{ic}")
                nc.sync.dma_start(
                    dtr[:, :, :], dd[:, :].rearrange("a (p f) -> p a f", p=128)
                )
                rtr = bcpool.tile([128, 2, 4], F32, tag="rtr", name=f"rtr{b}_{ic}")
                nc.vector.reciprocal(rtr[:, :, :], dtr[:, :, :])
                nc.sync.dma_start(
                    rd[:, :].rearrange("a (p f) -> p a f", p=128), rtr[:, :, :]
                )
                rec_bc = bcpool.tile([128, 512], F32, tag="rec_bc", name=f"rec_bc{b}_{ic}")
                for h in range(2):
                    nc.sync.dma_start(
                        rec_bc[h * HD : h * HD + HD, :],
                        rd[h : h + 1, :].broadcast_to([HD, 512]),
                    )
                st[("rec", ic)] = rec_bc

            def emit_norm_ic(b, ic):
                st = state[b]
                yTh = {0: st["yT0"], 1: st["yT1"]}
                rec_bc = st.pop(("rec", ic))
                st.pop(("den", ic))
                i0 = ic * 512
                yfp = ps_mm.tile([128, 512], F32, tag="mm", name=f"yfp{b}_{ic}")
                nc.tensor.matmul(
                    yfp[0:HD, :],
                    lhsT=ident[0:HD, 0:HD],
                    rhs=yTh[0][:, i0 : i0 + 512],
                    start=True,
                    stop=True,
                )
                nc.tensor.matmul(
                    yfp[HD:128, :],
                    lhsT=ident[0:HD, 0:HD],
                    rhs=yTh[1][:, i0 : i0 + 512],
                    start=True,
                    stop=True,
                    tile_position=(0, HD),
                )
                ynorm = spool.tile([128, 512], BF16, tag="ynorm", name=f"ynorm{b}_{ic}")
                nc.vector.tensor_tensor(
                    ynorm[:, :], yfp[:, :], rec_bc[:, :], op=OP.mult
                )
                st[("ynorm", ic)] = ynorm

            def emit_proj_ic(b, ic):
                st = state[b]
                tok0 = b * T
                ynorm = st.pop(("ynorm", ic))
                i0 = ic * 512
                fw = 512
                for tt in range(4):
                    tr0 = i0 + tt * 128
                    for fc in range(C // fw):
                        pp = ps_mm.tile([128, fw], F32, tag="mm", name=f"pp{b}_{ic}_{tt}_{fc}")
                        nc.tensor.matmul(
                            pp[:, :],
                            lhsT=ynorm[:, tt * 128 : tt * 128 + 128],
                            rhs=wp2_sb[:, fc * fw : fc * fw + fw],
                            start=True,
                            stop=True,
                        )
                        ot = opool.tile([128, fw], BF16, tag="ot", name=f"ot{b}_{ic}_{tt}_{fc}")
                        if (tt * 2 + fc) % 4 == 3:
                            nc.scalar.activation(ot[:, :], pp[:, :], AF.Copy)
                        else:
                            nc.vector.tensor_copy(ot[:, :], pp[:, :])
                        nc.sync.dma_start(
                            outp[
                                tok0 + tr0 : tok0 + tr0 + 128, fc * fw : fc * fw + fw
                            ],
                            ot[:, :],
                        )

            # ---- software-pipelined emission: qkv(b+1) interleaves with
            # attention(b), weighted toward the bigger i-chunks; proj lags its
            # i-chunk by one (norm early, matmuls late) so the reciprocal's
            # DMA round-trip and the DVE normalize are done by the time the
            # PE needs fill-in work between attention stalls ----
            alloc_qkv(0)
            for cn in range(NCH):
                emit_qkv_chunk(0, cn)
            prev = None
            for b in range(B):
                alloc_attn(b)
                if b + 1 < B:
                    alloc_qkv(b + 1)
                for ic in range(NCH):
                    emit_attn_ic(b, ic)
                    if b + 1 < B:
                        emit_qkv_chunk(b + 1, ic)
                    emit_recip_ic(b, ic)
                    if prev is not None:
                        emit_norm_ic(*prev)
                        emit_proj_ic(*prev)
                    prev = (b, ic)
            emit_norm_ic(*prev)
            emit_proj_ic(*prev)
    if split:
        _split_waits(nc)
    return nc


# ---------------------------------------------------------------- host side


def make_tables(T):
    inv_freq = 1.0 / (ROPE_BASE ** (np.arange(0, HD, 2, dtype=np.float32) / HD))
    pos = np.arange(T, dtype=np.float32)
    freqs = pos[:, None] * inv_freq[None, :]  # [T, 32]
    cos = np.cos(freqs).astype(np.float32)  # [T, 32] (same for both halves)
    sin = np.sin(freqs).astype(np.float32)
    cosT64 = np.concatenate([cos.T, cos.T], axis=0)  # [64, T]
    sinT64 = np.concatenate([-sin.T, sin.T], axis=0)  # sign-baked rotate_half
    cosT = np.concatenate([cosT64, cosT64], axis=0).copy()  # [128, T] two heads
    sinT = np.concatenate([sinT64, sinT64], axis=0).copy()
    return cosT, sinT


def make_mask4():
    # mask4[p, v*1024 + u*512 + f] = 1.0 if v*128 + p <= f else 0.0 (u: the
    # two heads sharing one [128,1024] exp tile, same diagonal mask each)
    m = np.zeros((128, 4 * 1024), dtype=np.float32)
    p = np.arange(128)[:, None]
    f = np.arange(512)[None, :]
    for v in range(4):
        blk = (v * 128 + p <= f).astype(np.float32)
        m[:, v * 1024 : v * 1024 + 512] = blk
        m[:, v * 1024 + 512 : v * 1024 + 1024] = blk
    return m


def make_in_maps(x, W_qkv, b_qkv, W_proj, n_cores):
    B, T, C = x.shape
    KC = C // 128
    import ml_dtypes

    # [global 512-token chunk, partition, KC, 512] so each chunk DMA reads
    # 8KB contiguous per partition
    xT = x.reshape(B * T, C).T.astype(ml_dtypes.bfloat16)
    xTc = np.ascontiguousarray(
        xT.reshape(KC, 128, B * T // 512, 512).transpose(2, 1, 0, 3)
    )

    def wlay(w):  # [C, 128] -> [128, KC*128] partition-major contiguous
        return np.ascontiguousarray(
            w.astype(ml_dtypes.bfloat16).reshape(KC, 128, 128).transpose(1, 0, 2)
            .reshape(128, KC * 128)
        )

    cosT, sinT = make_tables(T)
    cosT = cosT.astype(ml_dtypes.bfloat16)
    sinT = sinT.astype(ml_dtypes.bfloat16)
    mask4 = make_mask4().astype(ml_dtypes.bfloat16)
    in_maps = []
    for c in range(n_cores):
        h0 = 2 * c * HD  # first head's column offset (2 heads per core)
        sl = slice(h0, h0 + 128)
        in_maps.append(
            {
                "xTc": xTc,
                "wq": wlay(W_qkv[:, sl]),
                "wk": wlay(W_qkv[:, C:][:, sl]),
                "wv": wlay(W_qkv[:, 2 * C :][:, sl]),
                "bq": np.ascontiguousarray(b_qkv[sl].reshape(128, 1)),
                "bk": np.ascontiguousarray(b_qkv[C:][sl].reshape(128, 1)),
                "bv": np.ascontiguousarray(b_qkv[2 * C :][sl].reshape(128, 1)),
                "wp": np.ascontiguousarray(W_proj[sl, :].astype(ml_dtypes.bfloat16)),
                "cosT": cosT,
                "sinT": sinT,
                "mask4": mask4,
            }
        )
    return in_maps


_NC_CACHE = {}


def _get_nc(B, T, C):
    key = (B, T, C)
    if key not in _NC_CACHE:
        _NC_CACHE[key] = build_nc(B, T, C)
    return _NC_CACHE[key]


def kernel(x, W_qkv, b_qkv, W_proj, b_proj):
    from concourse.bass_utils import run_bass_kernel_spmd

    x = np.asarray(x, dtype=np.float32)
    W_qkv = np.asarray(W_qkv, dtype=np.float32)
    b_qkv = np.asarray(b_qkv, dtype=np.float32)
    W_proj = np.asarray(W_proj, dtype=np.float32)
    b_proj = np.asarray(b_proj, dtype=np.float32)
    B, T, C = x.shape
    n_cores = 8
    nc = _get_nc(B, T, C)
    in_maps = make_in_maps(x, W_qkv, b_qkv, W_proj, n_cores)
    res = run_bass_kernel_spmd(nc, in_maps, core_ids=list(range(n_cores)))
    out = np.zeros((B * T, C), dtype=np.float32)
    for r in res.results:
        out += r["outp"].astype(np.float32)
    out += b_proj[None, :]
    return out.reshape(B, T, C)


# revision 58
# speedup vs baseline: 1.1253x; 1.1253x over previous
"""Causal self-attention (RoPE) Trainium2 kernel, 8-way head-parallel.

Sharding: each of the 8 cores computes 2 of the 16 heads for all 4 batches
(tensor parallel over heads: W_qkv column-split, W_proj row-split). Host
pre-transposes x -> xT [C, B*T], slices per-core weights, and sum-reduces the
8 partial projection outputs (+ b_proj) — the standard row-parallel TP reduce.

Per-core dataflow (fp32 storage, bf16 matmuls):
  qkvT = W_slice.T @ xT            [feat, tok] PSUM, bias added on evac
  RoPE on qT,kT                    (rotate-half via SBUF partition-move DMAs)
  v: PE-transpose vT -> v tiles    [tok, d] (+ ones column for softmax denom)
  per (b, h, i-chunk):  S^T tiles = kT_jtile.T @ qT_ichunk   (j on partitions)
      two heads' S matmuls emitted as adjacent pairs -> PE row-tile
      concurrency ((0,0)/(64,0), K=64 each)
      P^T = exp(S^T/8) (ACT), causal mask on diagonal tiles (multiplicative,
      batched [128,1024] ops)
      [yT_h | denom] += v_aug.T @ P^T   accumulated over j-tiles in PSUM
  per i-chunk: denom -> gpsimd partition_broadcast -> DVE approx-reciprocal,
      yT merged via col-tiled identity matmuls, scaled, projected with
      N=1024 bf16-PSUM matmuls, written straight to outp rows.
"""

import numpy as np

import concourse.bass as bass
import concourse.mybir as mybir
import concourse.tile as tile

F32 = mybir.dt.float32
F32R = mybir.dt.float32r
BF16 = mybir.dt.bfloat16
AF = mybir.ActivationFunctionType
OP = mybir.AluOpType

# ---------------------------------------------------------------- tile patch
# This walrus build rejects >1 embedded sync-wait on sync-engine CTRL
# instructions; Tile's tail drain embeds one wait per outstanding semaphore.
# Split them across NOPs (1 wait each) before the drain.


def _patched_drain_and_barrier(self, tick_clock, wait_clock):
    from concourse.tile import ScopedClock

    nc = self.nc
    probe = nc.sync.nop(nofuse=True)
    wait_clock.add_sem_waits(probe.ins, ScopedClock({None: tick_clock.global_clock}))
    si = probe.ins.sync_info
    waits = list(si.on_wait) if si is not None and si.on_wait else []
    if len(waits) > 1:
        si.on_wait = waits[:1]
        for w in waits[1:]:
            nop = nc.sync.nop(nofuse=True)
            nsi = nop.ins.sync_info
            if nsi is None:
                nop.ins.sync_info = mybir.SyncInfo(on_wait=[w], on_update=[])
            else:
                nsi.on_wait = [w]
    nc.sync.drain()
    nc.all_engine_barrier()
    assert self.sems is not None
    popped = nc._tile_sem_poison_stack.pop()
    assert popped is self._sem_poison
    # chunk the sem clears: the range-encoded gpsimd drain (dma_reset) in this
    # walrus build rejects wide semaphore ranges ("ISA wrong length")
    sems = sorted(
        s.num if hasattr(s, "num") else s for s in self.sems.allocated().values()
    )
    for i in range(0, len(sems), 16):
        nc.clear_and_free_semaphores(sems[i : i + 16])
    nc.all_engine_barrier()


tile.TileContext._drain_and_barrier = _patched_drain_and_barrier


def _split_waits(nc):
    """Hoist all but one sync-wait per instruction onto same-engine NOPs
    (this walrus codegen supports a single embedded wait per instruction)."""
    n = 0
    for f in nc.m.functions:
        for blk in f.blocks:
            out = []
            changed = False
            for ins in blk.instructions:
                si = ins.sync_info
                if si is not None and si.on_wait and len(si.on_wait) > 1:
                    waits = list(si.on_wait)
                    for w in waits[:-1]:
                        n += 1
                        out.append(
                            mybir.InstNoOp(
                                name=f"wsplit{n}",
                                engine=ins.engine,
                                sync_info=mybir.SyncInfo(on_wait=[w], on_update=[]),
                                bass_nofuse=True,
                            )
                        )
                    si.on_wait = waits[-1:]
                    changed = True
                out.append(ins)
            if changed:
                blk.instructions = out
    return n

# ------------------------------------------------------------------- builder

HD = 64  # head dim (fixed)
ROPE_BASE = 10000.0


def build_nc(B, T, C, rope_k_engine="gpsimd", mask_engine="vector", split=True):
    """One core's program: 2 heads x B batches. T % 512 == 0, C % 128 == 0."""
    assert T % 512 == 0 and C % 128 == 0
    TOK = B * T
    KC = C // 128   # contraction chunks for QKV
    NCH = T // 512  # i-chunks per batch
    NJT = T // 128  # j-tiles per batch
    FW = 128        # qkv feature width per tensor (2 heads * 64)

    nc = bass.Bass()
    # x pre-arranged host-side as [global chunk, partition, KC*512] so each
    # chunk DMA is 128 x 8KB contiguous lines; weights as [128, KC*FW].
    xTc = nc.dram_tensor("xTc", [TOK // 512, 128, KC, 512], BF16, kind="ExternalInput")
    wq = nc.dram_tensor("wq", [128, KC * FW], BF16, kind="ExternalInput")
    wk = nc.dram_tensor("wk", [128, KC * FW], BF16, kind="ExternalInput")
    wv = nc.dram_tensor("wv", [128, KC * FW], BF16, kind="ExternalInput")
    bq = nc.dram_tensor("bq", [FW, 1], F32, kind="ExternalInput")
    bk = nc.dram_tensor("bk", [FW, 1], F32, kind="ExternalInput")
    bv = nc.dram_tensor("bv", [FW, 1], F32, kind="ExternalInput")
    wp = nc.dram_tensor("wp", [FW, C], BF16, kind="ExternalInput")
    cosT = nc.dram_tensor("cosT", [FW, T], BF16, kind="ExternalInput")
    sinT = nc.dram_tensor("sinT", [FW, T], BF16, kind="ExternalInput")
    mask4 = nc.dram_tensor("mask4", [128, 4 * 1024], BF16, kind="ExternalInput")
    outp = nc.dram_tensor("outp", [TOK, C], BF16, kind="ExternalOutput")

    def xchunk(gcn):
        return xTc[gcn : gcn + 1, :, :, :].rearrange("g p a f -> p (g a) f")

    rope_eng = {"vector": nc.vector, "gpsimd": nc.gpsimd}[rope_k_engine]
    mask_eng = {"vector": nc.vector, "gpsimd": nc.gpsimd}[mask_engine]

    with tile.TileContext(nc) as tc:
        with (
            tc.tile_pool(name="const", bufs=1) as cpool,
            tc.tile_pool(name="xt", bufs=4) as xpool,
            tc.tile_pool(name="qk", bufs=2) as qkpool,
            tc.tile_pool(name="vv", bufs=2) as vpool,
            tc.tile_pool(name="yy", bufs=2) as ypool,
            tc.tile_pool(name="small", bufs=3) as spool,
            tc.tile_pool(name="den", bufs=4) as dpool,
            tc.tile_pool(name="bc", bufs=3) as bcpool,
            tc.tile_pool(name="pt", bufs=6) as ptpool,
            tc.tile_pool(name="outs", bufs=6) as opool,
            tc.tile_pool(name="dram", bufs=2, space="DRAM") as drampool,
            tc.tile_pool(name="ps_mm", bufs=2, space="PSUM") as ps_mm,
            tc.tile_pool(name="ps_s", bufs=2, space="PSUM") as ps_s,
            tc.tile_pool(name="ps_y", bufs=1, space="PSUM") as ps_y,
        ):
            # ---- constants (small ones first; the big cos/sin/wp2 loads are
            # emitted after the first xt prefetches so they don't delay the
            # first QKV matmuls in the DMA queues) ----
            w_sb = {}
            for name, dram in (("wq", wq), ("wk", wk), ("wv", wv)):
                t = cpool.tile([128, KC, FW], BF16, tag=name)
                nc.sync.dma_start(
                    t[:, :, :], dram[:, :].rearrange("p (a f) -> p a f", a=KC)
                )
                w_sb[name] = t
            b_sb = {}
            for name, dram in (("bq", bq), ("bk", bk), ("bv", bv)):
                t = cpool.tile([FW, 1], F32, tag=name)
                nc.sync.dma_start(t[:, :], dram[:, :])
                b_sb[name] = t
            ident = cpool.tile([128, 128], BF16, tag="ident")
            from concourse.masks import make_identity

            make_identity(nc, ident[:, :])
            ones_raw = cpool.tile([128, 128], F32, tag="ones_raw")
            nc.vector.memset(ones_raw[:, :], 1.0)

            # prefetch the first batch's x chunks ahead of the big constants;
            # the first two are split across several DMA queues so the very
            # first QKV matmul isn't gated on a single-queue 1MB transfer
            xt_pre = {}
            for cn in range(min(NCH, 3)):
                xt = xpool.tile([128, KC, 512], BF16, tag="xt", name=f"xt0_{cn}")
                src = xchunk(cn)
                if cn < 2:
                    for q in range(4):
                        a0 = q * (KC // 4)
                        a1 = a0 + KC // 4
                        nc.sync.dma_start(xt[:, a0:a1, :], src[:, a0:a1, :])
                else:
                    nc.sync.dma_start(xt[:, :, :], src)
                xt_pre[cn] = xt

            cos_sb = cpool.tile([FW, T], BF16, tag="cos")
            nc.sync.dma_start(cos_sb[:, :], cosT[:, :])
            sin_sb = cpool.tile([FW, T], BF16, tag="sin")
            nc.sync.dma_start(sin_sb[:, :], sinT[:, :])
            mask_sb = cpool.tile([128, 4 * 1024], BF16, tag="mask")
            nc.sync.dma_start(mask_sb[:, :], mask4[:, :])
            wp2_sb = cpool.tile([FW, C], BF16, tag="wp2")
            nc.sync.dma_start(wp2_sb[:, :], wp[:, :])

            state = {}

            def alloc_qkv(b):
                st = state.setdefault(b, {})
                st["qT"] = qkpool.tile([FW, T], BF16, tag="qT", name=f"qT{b}")
                st["kT"] = qkpool.tile([FW, T], BF16, tag="kT", name=f"kT{b}")
                st["v0"] = vpool.tile([128, NJT, HD + 1], BF16, tag="v0", name=f"v0{b}")
                st["v1"] = vpool.tile([128, NJT, HD + 1], BF16, tag="v1", name=f"v1{b}")
                nc.vector.tensor_copy(
                    st["v0"][:, :, HD], ones_raw[:, 0:1].broadcast_to([128, NJT])
                )
                nc.vector.tensor_copy(
                    st["v1"][:, :, HD], ones_raw[:, 0:1].broadcast_to([128, NJT])
                )

            def emit_rope(b, cn, name, ps):
                st = state[b]
                ts0 = cn * 512
                dest = st["qT"] if name == "wq" else st["kT"]
                dch = dest[:, ts0 : ts0 + 512]
                bias = b_sb["bq" if name == "wq" else "bk"]
                nc.vector.tensor_scalar_add(dch, ps[:, :], bias[:, :])
                # rotate-half as 4 SBUF->SBUF DMA partition moves (sign is
                # baked into sinT); no PE/PSUM involved
                swp = spool.tile([128, 512], BF16, tag="swp", name=f"swp{b}_{cn}_{name}")
                for r0, r1 in ((0, 32), (32, 0), (64, 96), (96, 64)):
                    nc.sync.dma_start(
                        swp[r0 : r0 + 32, :], dch[r1 : r1 + 32, :]
                    )
                cc = cos_sb[:, ts0 : ts0 + 512]
                ss = sin_sb[:, ts0 : ts0 + 512]
                t1 = spool.tile([128, 512], F32, tag="t1", name=f"t1{b}_{cn}_{name}")
                t2 = spool.tile([128, 512], F32, tag="t2", name=f"t2{b}_{cn}_{name}")
                # b0's chunks run before attention exists: DVE is the pacing
                # engine there, so push the q-rope to gpsimd as well
                eng = nc.vector if (name == "wq" and b != 0) else rope_eng
                eng.tensor_tensor(t1[:, :], dch, cc, op=OP.mult)
                nc.vector.tensor_tensor(t2[:, :], swp[:, :], ss, op=OP.mult)
                eng.tensor_tensor(dch, t1[:, :], t2[:, :], op=OP.add)

            def emit_qkv_chunk(b, cn):
                st = state[b]
                v0, v1 = st["v0"], st["v1"]
                gcn = b * NCH + cn
                if b == 0 and cn in xt_pre:
                    xt = xt_pre.pop(cn)
                else:
                    xt = xpool.tile([128, KC, 512], BF16, tag="xt", name=f"xt{b}_{cn}")
                    nc.sync.dma_start(xt[:, :, :], xchunk(gcn))
                pss = {}
                for name in ("wq", "wk", "wv"):
                    ps = ps_mm.tile([128, 512], F32, tag="mm", name=f"qkvps{b}_{cn}_{name}")
                    for kc in range(KC):
                        nc.tensor.matmul(
                            ps[:, :],
                            lhsT=w_sb[name][:, kc, :],
                            rhs=xt[:, kc, :],
                            start=(kc == 0),
                            stop=(kc == KC - 1),
                        )
                    pss[name] = ps
                    if name == "wq":
                        # emitted after the wk chain is queued so the DVE evac
                        # overlaps PE work
                        pass
                    elif name == "wk":
                        emit_rope(b, cn, "wq", pss["wq"])
                    else:
                        emit_rope(b, cn, "wk", pss["wk"])
                ps = pss["wv"]
                vch = spool.tile([128, 512], BF16, tag="vch", name=f"vch{b}_{cn}")
                nc.vector.tensor_scalar_add(vch[:, :], ps[:, :], b_sb["bv"][:, :])
                # all four 128x128 transposes land in one PSUM tile so the v
                # evacuation is two wide strided copies
                pstw = ps_mm.tile([128, 512], BF16, tag="mm", name=f"pstw{b}_{cn}")
                for qd in range(4):
                    nc.tensor.transpose(
                        pstw[:, qd * 128 : qd * 128 + 128],
                        vch[:, qd * 128 : qd * 128 + 128],
                        ident[:, :],
                    )
                psv = pstw[:, :].rearrange("p (t f) -> p t f", t=4)
                nc.vector.tensor_copy(
                    v0[:, cn * 4 : cn * 4 + 4, 0:HD], psv[:, :, 0:HD]
                )
                nc.vector.tensor_copy(
                    v1[:, cn * 4 : cn * 4 + 4, 0:HD], psv[:, :, HD:128]
                )

            def alloc_attn(b):
                st = state[b]
                st["yT0"] = ypool.tile([HD, T], BF16, tag="yT0", name=f"yT0{b}")
                st["yT1"] = ypool.tile([HD, T], BF16, tag="yT1", name=f"yT1{b}")

            def emit_attn_ic(b, ic):
                st = state[b]
                qT, kT = st["qT"], st["kT"]
                vh = {0: st["v0"], 1: st["v1"]}
                yTh = {0: st["yT0"], 1: st["yT1"]}
                i0 = ic * 512
                njt = (ic + 1) * 4
                yps = {}
                for h in range(2):
                    yps[h] = ps_y.tile([HD + 1, 512], F32, tag=f"y{h}", name=f"yps{b}_{ic}_{h}")

                def emit_pv(jt, ptt):
                    for h in range(2):
                        nc.tensor.matmul(
                            yps[h][:, :],
                            lhsT=vh[h][:, jt, :],
                            rhs=ptt[:, h * 512 : h * 512 + 512],
                            start=(jt == 0),
                            stop=(jt == njt - 1),
                            skip_group_check=True,
                        )

                # Both heads' S for one j-tile share one [128,1024] sps tile
                # (two PSUM banks): the pair-mates then have the same
                # readiness gate, issue adjacently, and run concurrently on
                # PE row-tiles (0,0)/(64,0).
                for jt in range(njt):
                    sp = ps_s.tile([128, 1024], F32, tag="s", name=f"sps{b}_{ic}_{jt}")
                    for h in range(2):
                        hr0 = h * HD
                        nc.tensor.matmul(
                            sp[:, h * 512 : h * 512 + 512],
                            lhsT=kT[hr0 : hr0 + HD, jt * 128 : jt * 128 + 128],
                            rhs=qT[hr0 : hr0 + HD, i0 : i0 + 512],
                            start=True,
                            stop=True,
                        )
                    ptt = ptpool.tile([128, 1024], BF16, tag="pt", name=f"pt{b}_{ic}_{jt}")
                    nc.scalar.activation(
                        ptt[:, :],
                        sp[:, :],
                        AF.Exp,
                        scale=float(1.0 / np.sqrt(HD)),
                    )
                    dv = jt - (njt - 4)
                    if dv >= 0:  # diagonal tile: mask both heads at once
                        mask_eng.tensor_tensor(
                            ptt[:, :],
                            ptt[:, :],
                            mask_sb[:, dv * 1024 : dv * 1024 + 1024],
                            op=OP.mult,
                        )
                    emit_pv(jt, ptt)
                dts = {}
                for h in range(2):
                    nc.scalar.activation(
                        yTh[h][:, i0 : i0 + 512], yps[h][0:HD, :], AF.Copy
                    )
                    den_t = dpool.tile([HD + 1, 512], F32, tag="den", name=f"den{b}_{ic}_{h}")
                    nc.vector.tensor_copy(den_t[HD : HD + 1, :], yps[h][HD : HD + 1, :])
                    dts[h] = den_t
                st[("den", ic)] = dts

            def emit_recip_ic(b, ic):
                # reciprocal computed in a DMA-transposed [128, 8] layout so
                # all DVE lanes work (the natural [2, 512] layout would leave
                # 126 lanes idle for the slow iterative-divide op), then
                # DMA-broadcast from DRAM across the 64 partitions per head.
                st = state[b]
                dts = st[("den", ic)]
                dd = drampool.tile([2, 512], F32, tag="dd", name=f"dd{b}_{ic}")
                rd = drampool.tile([2, 512], F32, tag="rd", name=f"rd{b}_{ic}")
                for h in range(2):
                    nc.sync.dma_start(dd[h : h + 1, :], dts[h][HD : HD + 1, :])
                dtr = bcpool.tile([128, 2, 4], F32, tag="dtr", name=f"dtr{b}_{ic}")
                nc.sync.dma_start(
                    dtr[:, :, :], dd[:, :].rearrange("a (p f) -> p a f", p=128)
                )
                rtr = bcpool.tile([128, 2, 4], F32, tag="rtr", name=f"rtr{b}_{ic}")
                nc.vector.reciprocal(rtr[:, :, :], dtr[:, :, :])
                nc.sync.dma_start(
                    rd[:, :].rearrange("a (p f) -> p a f", p=128), rtr[:, :, :]
                )
                rec_bc = bcpool.tile([128, 512], F32, tag="rec_bc", name=f"rec_bc{b}_{ic}")
                for h in range(2):
                    nc.sync.dma_start(
                        rec_bc[h * HD : h * HD + HD, :],
                        rd[h : h + 1, :].broadcast_to([HD, 512]),
                    )
                st[("rec", ic)] = rec_bc

            def emit_norm_ic(b, ic):
                st = state[b]
                yTh = {0: st["yT0"], 1: st["yT1"]}
                rec_bc = st.pop(("rec", ic))
                st.pop(("den", ic))
                i0 = ic * 512
                yfp = ps_mm.tile([128, 512], F32, tag="mm", name=f"yfp{b}_{ic}")
                nc.tensor.matmul(
                    yfp[0:HD, :],
                    lhsT=ident[0:HD, 0:HD],
                    rhs=yTh[0][:, i0 : i0 + 512],
                    start=True,
                    stop=True,
                )
                nc.tensor.matmul(
                    yfp[HD:128, :],
                    lhsT=ident[0:HD, 0:HD],
                    rhs=yTh[1][:, i0 : i0 + 512],
                    start=True,
                    stop=True,
                    tile_position=(0, HD),
                )
                ynorm = spool.tile([128, 512], BF16, tag="ynorm", name=f"ynorm{b}_{ic}")
                nc.vector.tensor_tensor(
                    ynorm[:, :], yfp[:, :], rec_bc[:, :], op=OP.mult
                )
                st[("ynorm", ic)] = ynorm

            def emit_proj_ic(b, ic):
                st = state[b]
                tok0 = b * T
                ynorm = st.pop(("ynorm", ic))
                i0 = ic * 512
                fw = 512
                for tt in range(4):
                    tr0 = i0 + tt * 128
                    for fc in range(C // fw):
                        pp = ps_mm.tile([128, fw], F32, tag="mm", name=f"pp{b}_{ic}_{tt}_{fc}")
                        nc.tensor.matmul(
                            pp[:, :],
                            lhsT=ynorm[:, tt * 128 : tt * 128 + 128],
                            rhs=wp2_sb[:, fc * fw : fc * fw + fw],
                            start=True,
                            stop=True,
                        )
                        ot = opool.tile([128, fw], BF16, tag="ot", name=f"ot{b}_{ic}_{tt}_{fc}")
                        if (tt * 2 + fc) % 4 == 3:
                            nc.scalar.activation(ot[:, :], pp[:, :], AF.Copy)
                        else:
                            nc.vector.tensor_copy(ot[:, :], pp[:, :])
                        nc.sync.dma_start(
                            outp[
                                tok0 + tr0 : tok0 + tr0 + 128, fc * fw : fc * fw + fw
                            ],
                            ot[:, :],
                        )

            # ---- software-pipelined emission: qkv(b+1) interleaves with
            # attention(b), weighted toward the bigger i-chunks; proj lags its
            # i-chunk by one (norm early, matmuls late) so the reciprocal's
            # DMA round-trip and the DVE normalize are done by the time the
            # PE needs fill-in work between attention stalls ----
            alloc_qkv(0)
            for cn in range(NCH):
                emit_qkv_chunk(0, cn)
            prev = None
            for b in range(B):
                alloc_attn(b)
                if b + 1 < B:
                    alloc_qkv(b + 1)
                for ic in range(NCH):
                    emit_attn_ic(b, ic)
                    if b + 1 < B:
                        emit_qkv_chunk(b + 1, ic)
                    emit_recip_ic(b, ic)
                    if prev is not None:
                        emit_norm_ic(*prev)
                        emit_proj_ic(*prev)
                    prev = (b, ic)
            emit_norm_ic(*prev)
            emit_proj_ic(*prev)
    if split:
        _split_waits(nc)
    return nc


# ---------------------------------------------------------------- host side


def make_tables(T):
    inv_freq = 1.0 / (ROPE_BASE ** (np.arange(0, HD, 2, dtype=np.float32) / HD))
    pos = np.arange(T, dtype=np.float32)
    freqs = pos[:, None] * inv_freq[None, :]  # [T, 32]
    cos = np.cos(freqs).astype(np.float32)  # [T, 32] (same for both halves)
    sin = np.sin(freqs).astype(np.float32)
    cosT64 = np.concatenate([cos.T, cos.T], axis=0)  # [64, T]
    sinT64 = np.concatenate([-sin.T, sin.T], axis=0)  # sign-baked rotate_half
    cosT = np.concatenate([cosT64, cosT64], axis=0).copy()  # [128, T] two heads
    sinT = np.concatenate([sinT64, sinT64], axis=0).copy()
    return cosT, sinT


def make_mask4():
    # mask4[p, v*1024 + u*512 + f] = 1.0 if v*128 + p <= f else 0.0 (u: the
    # two heads sharing one [128,1024] exp tile, same diagonal mask each)
    m = np.zeros((128, 4 * 1024), dtype=np.float32)
    p = np.arange(128)[:, None]
    f = np.arange(512)[None, :]
    for v in range(4):
        blk = (v * 128 + p <= f).astype(np.float32)
        m[:, v * 1024 : v * 1024 + 512] = blk
        m[:, v * 1024 + 512 : v * 1024 + 1024] = blk
    return m


def make_in_maps(x, W_qkv, b_qkv, W_proj, n_cores):
    B, T, C = x.shape
    KC = C // 128
    import ml_dtypes

    # [global 512-token chunk, partition, KC, 512] so each chunk DMA reads
    # 8KB contiguous per partition
    xT = x.reshape(B * T, C).T.astype(ml_dtypes.bfloat16)
    xTc = np.ascontiguousarray(
        xT.reshape(KC, 128, B * T // 512, 512).transpose(2, 1, 0, 3)
    )

    def wlay(w):  # [C, 128] -> [128, KC*128] partition-major contiguous
        return np.ascontiguousarray(
            w.astype(ml_dtypes.bfloat16).reshape(KC, 128, 128).transpose(1, 0, 2)
            .reshape(128, KC * 128)
        )

    cosT, sinT = make_tables(T)
    cosT = cosT.astype(ml_dtypes.bfloat16)
    sinT = sinT.astype(ml_dtypes.bfloat16)
    mask4 = make_mask4().astype(ml_dtypes.bfloat16)
    in_maps = []
    for c in range(n_cores):
        h0 = 2 * c * HD  # first head's column offset (2 heads per core)
        sl = slice(h0, h0 + 128)
        in_maps.append(
            {
                "xTc": xTc,
                "wq": wlay(W_qkv[:, sl]),
                "wk": wlay(W_qkv[:, C:][:, sl]),
                "wv": wlay(W_qkv[:, 2 * C :][:, sl]),
                "bq": np.ascontiguousarray(b_qkv[sl].reshape(128, 1)),
                "bk": np.ascontiguousarray(b_qkv[C:][sl].reshape(128, 1)),
                "bv": np.ascontiguousarray(b_qkv[2 * C :][sl].reshape(128, 1)),
                "wp": np.ascontiguousarray(W_proj[sl, :].astype(ml_dtypes.bfloat16)),
                "cosT": cosT,
                "sinT": sinT,
                "mask4": mask4,
            }
        )
    return in_maps


_NC_CACHE = {}


def _get_nc(B, T, C):
    key = (B, T, C)
    if key not in _NC_CACHE:
        _NC_CACHE[key] = build_nc(B, T, C)
    return _NC_CACHE[key]


def kernel(x, W_qkv, b_qkv, W_proj, b_proj):
    from concourse.bass_utils import run_bass_kernel_spmd

    x = np.asarray(x, dtype=np.float32)
    W_qkv = np.asarray(W_qkv, dtype=np.float32)
    b_qkv = np.asarray(b_qkv, dtype=np.float32)
    W_proj = np.asarray(W_proj, dtype=np.float32)
    b_proj = np.asarray(b_proj, dtype=np.float32)
    B, T, C = x.shape
    n_cores = 8
    nc = _get_nc(B, T, C)
    in_maps = make_in_maps(x, W_qkv, b_qkv, W_proj, n_cores)
    res = run_bass_kernel_spmd(nc, in_maps, core_ids=list(range(n_cores)))
    out = np.zeros((B * T, C), dtype=np.float32)
    for r in res.results:
        out += r["outp"].astype(np.float32)
    out += b_proj[None, :]
    return out.reshape(B, T, C)


# revision 61
# speedup vs baseline: 1.1416x; 1.0145x over previous
"""Causal self-attention (RoPE) Trainium2 kernel, 8-way head-parallel.

Sharding: each of the 8 cores computes 2 of the 16 heads for all 4 batches
(tensor parallel over heads: W_qkv column-split, W_proj row-split). Host
pre-transposes x -> xT [C, B*T], slices per-core weights, and sum-reduces the
8 partial projection outputs (+ b_proj) — the standard row-parallel TP reduce.

Per-core dataflow (fp32 storage, bf16 matmuls):
  qkvT = W_slice.T @ xT            [feat, tok] PSUM, bias added on evac
  RoPE on qT,kT                    (rotate-half via SBUF partition-move DMAs)
  v: PE-transpose vT -> v tiles    [tok, d] (+ ones column for softmax denom)
  per (b, h, i-chunk):  S^T tiles = kT_jtile.T @ qT_ichunk   (j on partitions)
      two heads' S matmuls emitted as adjacent pairs -> PE row-tile
      concurrency ((0,0)/(64,0), K=64 each)
      P^T = exp(S^T/8) (ACT), causal mask on diagonal tiles (multiplicative,
      batched [128,1024] ops)
      [yT_h | denom] += v_aug.T @ P^T   accumulated over j-tiles in PSUM
  per i-chunk: denom -> gpsimd partition_broadcast -> DVE approx-reciprocal,
      yT merged via col-tiled identity matmuls, scaled, projected with
      N=1024 bf16-PSUM matmuls, written straight to outp rows.
"""

import numpy as np

import concourse.bass as bass
import concourse.mybir as mybir
import concourse.tile as tile

F32 = mybir.dt.float32
F32R = mybir.dt.float32r
BF16 = mybir.dt.bfloat16
AF = mybir.ActivationFunctionType
OP = mybir.AluOpType

# ---------------------------------------------------------------- tile patch
# This walrus build rejects >1 embedded sync-wait on sync-engine CTRL
# instructions; Tile's tail drain embeds one wait per outstanding semaphore.
# Split them across NOPs (1 wait each) before the drain.


def _patched_drain_and_barrier(self, tick_clock, wait_clock):
    from concourse.tile import ScopedClock

    nc = self.nc
    probe = nc.sync.nop(nofuse=True)
    wait_clock.add_sem_waits(probe.ins, ScopedClock({None: tick_clock.global_clock}))
    si = probe.ins.sync_info
    waits = list(si.on_wait) if si is not None and si.on_wait else []
    if len(waits) > 1:
        si.on_wait = waits[:1]
        for w in waits[1:]:
            nop = nc.sync.nop(nofuse=True)
            nsi = nop.ins.sync_info
            if nsi is None:
                nop.ins.sync_info = mybir.SyncInfo(on_wait=[w], on_update=[])
            else:
                nsi.on_wait = [w]
    nc.sync.drain()
    nc.all_engine_barrier()
    assert self.sems is not None
    popped = nc._tile_sem_poison_stack.pop()
    assert popped is self._sem_poison
    # chunk the sem clears: the range-encoded gpsimd drain (dma_reset) in this
    # walrus build rejects wide semaphore ranges ("ISA wrong length")
    sems = sorted(
        s.num if hasattr(s, "num") else s for s in self.sems.allocated().values()
    )
    for i in range(0, len(sems), 16):
        nc.clear_and_free_semaphores(sems[i : i + 16])
    nc.all_engine_barrier()


tile.TileContext._drain_and_barrier = _patched_drain_and_barrier


def _split_waits(nc):
    """Hoist all but one sync-wait per instruction onto same-engine NOPs
    (this walrus codegen supports a single embedded wait per instruction)."""
    n = 0
    for f in nc.m.functions:
        for blk in f.blocks:
            out = []
            changed = False
            for ins in blk.instructions:
                si = ins.sync_info
                if si is not None and si.on_wait and len(si.on_wait) > 1:
                    waits = list(si.on_wait)
                    for w in waits[:-1]:
                        n += 1
                        out.append(
                            mybir.InstNoOp(
                                name=f"wsplit{n}",
                                engine=ins.engine,
                                sync_info=mybir.SyncInfo(on_wait=[w], on_update=[]),
                                bass_nofuse=True,
                            )
                        )
                    si.on_wait = waits[-1:]
                    changed = True
                out.append(ins)
            if changed:
                blk.instructions = out
    return n

# ------------------------------------------------------------------- builder

HD = 64  # head dim (fixed)
ROPE_BASE = 10000.0


def build_nc(B, T, C, rope_k_engine="gpsimd", mask_engine="vector", split=True):
    """One core's program: 2 heads x B batches. T % 512 == 0, C % 128 == 0."""
    assert T % 512 == 0 and C % 128 == 0
    TOK = B * T
    KC = C // 128   # contraction chunks for QKV
    NCH = T // 512  # i-chunks per batch
    NJT = T // 128  # j-tiles per batch
    FW = 128        # qkv feature width per tensor (2 heads * 64)

    nc = bass.Bass()
    # x pre-arranged host-side as [global chunk, partition, KC*512] so each
    # chunk DMA is 128 x 8KB contiguous lines; weights as [128, KC*FW].
    xTc = nc.dram_tensor("xTc", [TOK // 512, 128, KC, 512], BF16, kind="ExternalInput")
    wq = nc.dram_tensor("wq", [128, KC * FW], BF16, kind="ExternalInput")
    wk = nc.dram_tensor("wk", [128, KC * FW], BF16, kind="ExternalInput")
    wv = nc.dram_tensor("wv", [128, KC * FW], BF16, kind="ExternalInput")
    bq = nc.dram_tensor("bq", [FW, 1], F32, kind="ExternalInput")
    bk = nc.dram_tensor("bk", [FW, 1], F32, kind="ExternalInput")
    bv = nc.dram_tensor("bv", [FW, 1], F32, kind="ExternalInput")
    wp = nc.dram_tensor("wp", [FW, C], BF16, kind="ExternalInput")
    cosT = nc.dram_tensor("cosT", [FW, T], BF16, kind="ExternalInput")
    sinT = nc.dram_tensor("sinT", [FW, T], BF16, kind="ExternalInput")
    mask4 = nc.dram_tensor("mask4", [128, 4 * 1024], BF16, kind="ExternalInput")
    outp = nc.dram_tensor("outp", [TOK, C], BF16, kind="ExternalOutput")

    def xchunk(gcn):
        return xTc[gcn : gcn + 1, :, :, :].rearrange("g p a f -> p (g a) f")

    rope_eng = {"vector": nc.vector, "gpsimd": nc.gpsimd}[rope_k_engine]
    mask_eng = {"vector": nc.vector, "gpsimd": nc.gpsimd}[mask_engine]

    with tile.TileContext(nc) as tc:
        with (
            tc.tile_pool(name="const", bufs=1) as cpool,
            tc.tile_pool(name="xt", bufs=5) as xpool,
            tc.tile_pool(name="qk", bufs=2) as qkpool,
            tc.tile_pool(name="vv", bufs=2) as vpool,
            tc.tile_pool(name="yy", bufs=2) as ypool,
            tc.tile_pool(name="small", bufs=4) as spool,
            tc.tile_pool(name="den", bufs=4) as dpool,
            tc.tile_pool(name="bc", bufs=3) as bcpool,
            tc.tile_pool(name="pt", bufs=6) as ptpool,
            tc.tile_pool(name="outs", bufs=6) as opool,
            tc.tile_pool(name="dram", bufs=4, space="DRAM") as drampool,
            tc.tile_pool(name="ps_mm", bufs=2, space="PSUM") as ps_mm,
            tc.tile_pool(name="ps_s", bufs=2, space="PSUM") as ps_s,
            tc.tile_pool(name="ps_y", bufs=1, space="PSUM") as ps_y,
        ):
            # ---- constants (small ones first; the big cos/sin/wp2 loads are
            # emitted after the first xt prefetches so they don't delay the
            # first QKV matmuls in the DMA queues) ----
            w_sb = {}
            for name, dram in (("wq", wq), ("wk", wk), ("wv", wv)):
                t = cpool.tile([128, KC, FW], BF16, tag=name)
                nc.sync.dma_start(
                    t[:, :, :], dram[:, :].rearrange("p (a f) -> p a f", a=KC)
                )
                w_sb[name] = t
            b_sb = {}
            for name, dram in (("bq", bq), ("bk", bk), ("bv", bv)):
                t = cpool.tile([FW, 1], F32, tag=name)
                nc.sync.dma_start(t[:, :], dram[:, :])
                b_sb[name] = t
            ident = cpool.tile([128, 128], BF16, tag="ident")
            from concourse.masks import make_identity

            make_identity(nc, ident[:, :])
            ones_raw = cpool.tile([128, 128], F32, tag="ones_raw")
            nc.vector.memset(ones_raw[:, :], 1.0)

            # prefetch the first batch's x chunks ahead of the big constants;
            # the first two are split across several DMA queues so the very
            # first QKV matmul isn't gated on a single-queue 1MB transfer
            xt_pre = {}
            for cn in range(min(NCH, 3)):
                xt = xpool.tile([128, KC, 512], BF16, tag="xt", name=f"xt0_{cn}")
                src = xchunk(cn)
                if cn < 2:
                    for q in range(4):
                        a0 = q * (KC // 4)
                        a1 = a0 + KC // 4
                        nc.sync.dma_start(xt[:, a0:a1, :], src[:, a0:a1, :])
                else:
                    nc.sync.dma_start(xt[:, :, :], src)
                xt_pre[cn] = xt

            cos_sb = cpool.tile([FW, T], BF16, tag="cos")
            nc.sync.dma_start(cos_sb[:, :], cosT[:, :])
            sin_sb = cpool.tile([FW, T], BF16, tag="sin")
            nc.sync.dma_start(sin_sb[:, :], sinT[:, :])
            mask_sb = cpool.tile([128, 4 * 1024], BF16, tag="mask")
            nc.sync.dma_start(mask_sb[:, :], mask4[:, :])
            wp2_sb = cpool.tile([FW, C], BF16, tag="wp2")
            nc.sync.dma_start(wp2_sb[:, :], wp[:, :])

            state = {}

            def alloc_qkv(b):
                st = state.setdefault(b, {})
                st["qT"] = qkpool.tile([FW, T], BF16, tag="qT", name=f"qT{b}")
                st["kT"] = qkpool.tile([FW, T], BF16, tag="kT", name=f"kT{b}")
                st["v0"] = vpool.tile([128, NJT, HD + 1], BF16, tag="v0", name=f"v0{b}")
                st["v1"] = vpool.tile([128, NJT, HD + 1], BF16, tag="v1", name=f"v1{b}")
                nc.vector.tensor_copy(
                    st["v0"][:, :, HD], ones_raw[:, 0:1].broadcast_to([128, NJT])
                )
                nc.vector.tensor_copy(
                    st["v1"][:, :, HD], ones_raw[:, 0:1].broadcast_to([128, NJT])
                )

            def emit_rope(b, cn, name, ps):
                st = state[b]
                ts0 = cn * 512
                dest = st["qT"] if name == "wq" else st["kT"]
                dch = dest[:, ts0 : ts0 + 512]
                bias = b_sb["bq" if name == "wq" else "bk"]
                nc.vector.tensor_scalar_add(dch, ps[:, :], bias[:, :])
                # rotate-half as 4 SBUF->SBUF DMA partition moves (sign is
                # baked into sinT); no PE/PSUM involved
                swp = spool.tile([128, 512], BF16, tag="swp", name=f"swp{b}_{cn}_{name}")
                for r0, r1 in ((0, 32), (32, 0), (64, 96), (96, 64)):
                    nc.sync.dma_start(
                        swp[r0 : r0 + 32, :], dch[r1 : r1 + 32, :]
                    )
                cc = cos_sb[:, ts0 : ts0 + 512]
                ss = sin_sb[:, ts0 : ts0 + 512]
                t1 = spool.tile([128, 512], F32, tag="t1", name=f"t1{b}_{cn}_{name}")
                t2 = spool.tile([128, 512], F32, tag="t2", name=f"t2{b}_{cn}_{name}")
                # b0's chunks run before attention exists: DVE is the pacing
                # engine there, so push the q-rope to gpsimd as well
                eng = nc.vector if (name == "wq" and b != 0) else rope_eng
                eng.tensor_tensor(t1[:, :], dch, cc, op=OP.mult)
                nc.vector.tensor_tensor(t2[:, :], swp[:, :], ss, op=OP.mult)
                eng.tensor_tensor(dch, t1[:, :], t2[:, :], op=OP.add)

            def emit_qkv_chunk(b, cn):
                st = state[b]
                v0, v1 = st["v0"], st["v1"]
                gcn = b * NCH + cn
                if b == 0 and cn in xt_pre:
                    xt = xt_pre.pop(cn)
                else:
                    xt = xpool.tile([128, KC, 512], BF16, tag="xt", name=f"xt{b}_{cn}")
                    nc.sync.dma_start(xt[:, :, :], xchunk(gcn))
                pss = {}
                for name in ("wq", "wk", "wv"):
                    ps = ps_mm.tile([128, 512], F32, tag="mm", name=f"qkvps{b}_{cn}_{name}")
                    for kc in range(KC):
                        nc.tensor.matmul(
                            ps[:, :],
                            lhsT=w_sb[name][:, kc, :],
                            rhs=xt[:, kc, :],
                            start=(kc == 0),
                            stop=(kc == KC - 1),
                        )
                    pss[name] = ps
                    if name == "wq":
                        # emitted after the wk chain is queued so the DVE evac
                        # overlaps PE work
                        pass
                    elif name == "wk":
                        emit_rope(b, cn, "wq", pss["wq"])
                    else:
                        emit_rope(b, cn, "wk", pss["wk"])
                ps = pss["wv"]
                vch = spool.tile([128, 512], BF16, tag="vch", name=f"vch{b}_{cn}")
                nc.vector.tensor_scalar_add(vch[:, :], ps[:, :], b_sb["bv"][:, :])
                # all four 128x128 transposes land in one PSUM tile so the v
                # evacuation is two wide strided copies
                pstw = ps_mm.tile([128, 512], BF16, tag="mm", name=f"pstw{b}_{cn}")
                for qd in range(4):
                    nc.tensor.transpose(
                        pstw[:, qd * 128 : qd * 128 + 128],
                        vch[:, qd * 128 : qd * 128 + 128],
                        ident[:, :],
                    )
                psv = pstw[:, :].rearrange("p (t f) -> p t f", t=4)
                nc.vector.tensor_copy(
                    v0[:, cn * 4 : cn * 4 + 4, 0:HD], psv[:, :, 0:HD]
                )
                nc.vector.tensor_copy(
                    v1[:, cn * 4 : cn * 4 + 4, 0:HD], psv[:, :, HD:128]
                )

            def alloc_attn(b):
                st = state[b]
                st["yT0"] = ypool.tile([HD, T], BF16, tag="yT0", name=f"yT0{b}")
                st["yT1"] = ypool.tile([HD, T], BF16, tag="yT1", name=f"yT1{b}")

            def emit_attn_ic(b, ic):
                st = state[b]
                qT, kT = st["qT"], st["kT"]
                vh = {0: st["v0"], 1: st["v1"]}
                yTh = {0: st["yT0"], 1: st["yT1"]}
                i0 = ic * 512
                njt = (ic + 1) * 4
                yps = {}
                for h in range(2):
                    yps[h] = ps_y.tile([HD + 1, 512], F32, tag=f"y{h}", name=f"yps{b}_{ic}_{h}")

                def emit_pv(jt, ptt):
                    for h in range(2):
                        nc.tensor.matmul(
                            yps[h][:, :],
                            lhsT=vh[h][:, jt, :],
                            rhs=ptt[:, h * 512 : h * 512 + 512],
                            start=(jt == 0),
                            stop=(jt == njt - 1),
                            skip_group_check=True,
                        )

                # Both heads' S for one j-tile share one [128,1024] sps tile
                # (two PSUM banks): the pair-mates then have the same
                # readiness gate, issue adjacently, and run concurrently on
                # PE row-tiles (0,0)/(64,0).
                for jt in range(njt):
                    sp = ps_s.tile([128, 1024], F32, tag="s", name=f"sps{b}_{ic}_{jt}")
                    for h in range(2):
                        hr0 = h * HD
                        nc.tensor.matmul(
                            sp[:, h * 512 : h * 512 + 512],
                            lhsT=kT[hr0 : hr0 + HD, jt * 128 : jt * 128 + 128],
                            rhs=qT[hr0 : hr0 + HD, i0 : i0 + 512],
                            start=True,
                            stop=True,
                        )
                    ptt = ptpool.tile([128, 1024], BF16, tag="pt", name=f"pt{b}_{ic}_{jt}")
                    nc.scalar.activation(
                        ptt[:, :],
                        sp[:, :],
                        AF.Exp,
                        scale=float(1.0 / np.sqrt(HD)),
                    )
                    dv = jt - (njt - 4)
                    if dv >= 0:  # diagonal tile: mask both heads at once
                        mask_eng.tensor_tensor(
                            ptt[:, :],
                            ptt[:, :],
                            mask_sb[:, dv * 1024 : dv * 1024 + 1024],
                            op=OP.mult,
                        )
                    emit_pv(jt, ptt)
                dts = {}
                for h in range(2):
                    nc.scalar.activation(
                        yTh[h][:, i0 : i0 + 512], yps[h][0:HD, :], AF.Copy
                    )
                    den_t = dpool.tile([HD + 1, 512], F32, tag="den", name=f"den{b}_{ic}_{h}")
                    nc.vector.tensor_copy(den_t[HD : HD + 1, :], yps[h][HD : HD + 1, :])
                    dts[h] = den_t
                st[("den", ic)] = dts

            def emit_recip_ic(b, ic):
                # reciprocal computed in a DMA-transposed [128, 8] layout so
                # all DVE lanes work (the natural [2, 512] layout would leave
                # 126 lanes idle for the slow iterative-divide op), then
                # DMA-broadcast from DRAM across the 64 partitions per head.
                st = state[b]
                dts = st[("den", ic)]
                dd = drampool.tile([2, 512], F32, tag="dd", name=f"dd{b}_{ic}")
                rd = drampool.tile([2, 512], F32, tag="rd", name=f"rd{b}_{ic}")
                for h in range(2):
                    nc.sync.dma_start(dd[h : h + 1, :], dts[h][HD : HD + 1, :])
                dtr = bcpool.tile([128, 2, 4], F32, tag="dtr", name=f"dtr{b}_{ic}")
                nc.sync.dma_start(
                    dtr[:, :, :], dd[:, :].rearrange("a (p f) -> p a f", p=128)
                )
                rtr = bcpool.tile([128, 2, 4], F32, tag="rtr", name=f"rtr{b}_{ic}")
                nc.vector.reciprocal(rtr[:, :, :], dtr[:, :, :])
                nc.sync.dma_start(
                    rd[:, :].rearrange("a (p f) -> p a f", p=128), rtr[:, :, :]
                )
                rec_bc = bcpool.tile([128, 512], F32, tag="rec_bc", name=f"rec_bc{b}_{ic}")
                for h in range(2):
                    nc.sync.dma_start(
                        rec_bc[h * HD : h * HD + HD, :],
                        rd[h : h + 1, :].broadcast_to([HD, 512]),
                    )
                st[("rec", ic)] = rec_bc

            def emit_norm_ic(b, ic):
                st = state[b]
                yTh = {0: st["yT0"], 1: st["yT1"]}
                rec_bc = st.pop(("rec", ic))
                st.pop(("den", ic))
                i0 = ic * 512
                yfp = ps_mm.tile([128, 512], F32, tag="mm", name=f"yfp{b}_{ic}")
                nc.tensor.matmul(
                    yfp[0:HD, :],
                    lhsT=ident[0:HD, 0:HD],
                    rhs=yTh[0][:, i0 : i0 + 512],
                    start=True,
                    stop=True,
                )
                nc.tensor.matmul(
                    yfp[HD:128, :],
                    lhsT=ident[0:HD, 0:HD],
                    rhs=yTh[1][:, i0 : i0 + 512],
                    start=True,
                    stop=True,
                    tile_position=(0, HD),
                )
                ynorm = spool.tile([128, 512], BF16, tag="ynorm", name=f"ynorm{b}_{ic}")
                nc.vector.tensor_tensor(
                    ynorm[:, :], yfp[:, :], rec_bc[:, :], op=OP.mult
                )
                st[("ynorm", ic)] = ynorm

            def emit_proj_ic(b, ic):
                st = state[b]
                tok0 = b * T
                ynorm = st.pop(("ynorm", ic))
                i0 = ic * 512
                fw = 512
                for tt in range(4):
                    tr0 = i0 + tt * 128
                    for fc in range(C // fw):
                        pp = ps_mm.tile([128, fw], F32, tag="mm", name=f"pp{b}_{ic}_{tt}_{fc}")
                        nc.tensor.matmul(
                            pp[:, :],
                            lhsT=ynorm[:, tt * 128 : tt * 128 + 128],
                            rhs=wp2_sb[:, fc * fw : fc * fw + fw],
                            start=True,
                            stop=True,
                        )
                        ot = opool.tile([128, fw], BF16, tag="ot", name=f"ot{b}_{ic}_{tt}_{fc}")
                        if (tt * 2 + fc) % 4 == 3:
                            nc.scalar.activation(ot[:, :], pp[:, :], AF.Copy)
                        else:
                            nc.vector.tensor_copy(ot[:, :], pp[:, :])
                        nc.sync.dma_start(
                            outp[
                                tok0 + tr0 : tok0 + tr0 + 128, fc * fw : fc * fw + fw
                            ],
                            ot[:, :],
                        )

            # ---- software-pipelined emission: qkv(b+1) interleaves with
            # attention(b), weighted toward the bigger i-chunks; proj lags its
            # i-chunk by one (norm early, matmuls late) so the reciprocal's
            # DMA round-trip and the DVE normalize are done by the time the
            # PE needs fill-in work between attention stalls ----
            alloc_qkv(0)
            for cn in range(NCH):
                emit_qkv_chunk(0, cn)
            prev = None
            for b in range(B):
                alloc_attn(b)
                if b + 1 < B:
                    alloc_qkv(b + 1)
                for ic in range(NCH):
                    emit_attn_ic(b, ic)
                    if b + 1 < B:
                        emit_qkv_chunk(b + 1, ic)
                    emit_recip_ic(b, ic)
                    if prev is not None:
                        emit_norm_ic(*prev)
                        emit_proj_ic(*prev)
                    prev = (b, ic)
            emit_norm_ic(*prev)
            emit_proj_ic(*prev)
    if split:
        _split_waits(nc)
    return nc


# ---------------------------------------------------------------- host side


def make_tables(T):
    inv_freq = 1.0 / (ROPE_BASE ** (np.arange(0, HD, 2, dtype=np.float32) / HD))
    pos = np.arange(T, dtype=np.float32)
    freqs = pos[:, None] * inv_freq[None, :]  # [T, 32]
    cos = np.cos(freqs).astype(np.float32)  # [T, 32] (same for both halves)
    sin = np.sin(freqs).astype(np.float32)
    cosT64 = np.concatenate([cos.T, cos.T], axis=0)  # [64, T]
    sinT64 = np.concatenate([-sin.T, sin.T], axis=0)  # sign-baked rotate_half
    cosT = np.concatenate([cosT64, cosT64], axis=0).copy()  # [128, T] two heads
    sinT = np.concatenate([sinT64, sinT64], axis=0).copy()
    return cosT, sinT


def make_mask4():
    # mask4[p, v*1024 + u*512 + f] = 1.0 if v*128 + p <= f else 0.0 (u: the
    # two heads sharing one [128,1024] exp tile, same diagonal mask each)
    m = np.zeros((128, 4 * 1024), dtype=np.float32)
    p = np.arange(128)[:, None]
    f = np.arange(512)[None, :]
    for v in range(4):
        blk = (v * 128 + p <= f).astype(np.float32)
        m[:, v * 1024 : v * 1024 + 512] = blk
        m[:, v * 1024 + 512 : v * 1024 + 1024] = blk
    return m


def make_in_maps(x, W_qkv, b_qkv, W_proj, n_cores):
    B, T, C = x.shape
    KC = C // 128
    import ml_dtypes

    # [global 512-token chunk, partition, KC, 512] so each chunk DMA reads
    # 8KB contiguous per partition
    xT = x.reshape(B * T, C).T.astype(ml_dtypes.bfloat16)
    xTc = np.ascontiguousarray(
        xT.reshape(KC, 128, B * T // 512, 512).transpose(2, 1, 0, 3)
    )

    def wlay(w):  # [C, 128] -> [128, KC*128] partition-major contiguous
        return np.ascontiguousarray(
            w.astype(ml_dtypes.bfloat16).reshape(KC, 128, 128).transpose(1, 0, 2)
            .reshape(128, KC * 128)
        )

    cosT, sinT = make_tables(T)
    cosT = cosT.astype(ml_dtypes.bfloat16)
    sinT = sinT.astype(ml_dtypes.bfloat16)
    mask4 = make_mask4().astype(ml_dtypes.bfloat16)
    in_maps = []
    for c in range(n_cores):
        h0 = 2 * c * HD  # first head's column offset (2 heads per core)
        sl = slice(h0, h0 + 128)
        in_maps.append(
            {
                "xTc": xTc,
                "wq": wlay(W_qkv[:, sl]),
                "wk": wlay(W_qkv[:, C:][:, sl]),
                "wv": wlay(W_qkv[:, 2 * C :][:, sl]),
                "bq": np.ascontiguousarray(b_qkv[sl].reshape(128, 1)),
                "bk": np.ascontiguousarray(b_qkv[C:][sl].reshape(128, 1)),
                "bv": np.ascontiguousarray(b_qkv[2 * C :][sl].reshape(128, 1)),
                "wp": np.ascontiguousarray(W_proj[sl, :].astype(ml_dtypes.bfloat16)),
                "cosT": cosT,
                "sinT": sinT,
                "mask4": mask4,
            }
        )
    return in_maps


_NC_CACHE = {}


def _get_nc(B, T, C):
    key = (B, T, C)
    if key not in _NC_CACHE:
        _NC_CACHE[key] = build_nc(B, T, C)
    return _NC_CACHE[key]


def kernel(x, W_qkv, b_qkv, W_proj, b_proj):
    from concourse.bass_utils import run_bass_kernel_spmd

    x = np.asarray(x, dtype=np.float32)
    W_qkv = np.asarray(W_qkv, dtype=np.float32)
    b_qkv = np.asarray(b_qkv, dtype=np.float32)
    W_proj = np.asarray(W_proj, dtype=np.float32)
    b_proj = np.asarray(b_proj, dtype=np.float32)
    B, T, C = x.shape
    n_cores = 8
    nc = _get_nc(B, T, C)
    in_maps = make_in_maps(x, W_qkv, b_qkv, W_proj, n_cores)
    res = run_bass_kernel_spmd(nc, in_maps, core_ids=list(range(n_cores)))
    out = np.zeros((B * T, C), dtype=np.float32)
    for r in res.results:
        out += r["outp"].astype(np.float32)
    out += b_proj[None, :]
    return out.reshape(B, T, C)
